# revision 12
# baseline (speedup 1.0000x reference)
"""Trainium2 Bass kernel for nn_LSTMClassifier (B=256,T=1024,D=64,H=128,C=10).

Strategy: data-parallel over batch across 8 cores (32 seqs/core).
Per-core layout is gate-major: partitions = hidden units, batch on the
free dim, so h^T [128,32] is born in the right layout to be the moving
operand of the next step's gate matmuls (no per-step transposes).

The two LSTM layers are MERGED into one instruction stream: at "pair"
m, layer 0 runs step t=m while layer 1 runs step t=m-LAGS, and every
activation / vector op covers both layers in a single instruction
(per-instruction fixed cost on ACT ~177-250ns and DVE ~155ns dwarfs
the per-element cost, so 64-wide ops are ~2x cheaper than 2x 32-wide).
PSUM bank columns are step-major and layer-interleaved:
    col = (m%2)*256 + gate*64 + layer*32 + b
so sigmoid(i,f), sigmoid(o), tanh(g) and all DVE ops read/write
contiguous column ranges covering both layers at once.

Per layer the input transform xg = W_ih @ x + b is computed by chunked
GEMMs directly INTO the PSUM banks that the per-step recurrence matmuls
then accumulate onto (start=False). L0's bias rides a K=65 augmented
stationary (ones row in x^T); L1's bias is one indicator matmul per
bank (also doubling as the bank's start=True clear).
"""

import os
import sys

import numpy as np

for _p in ("/opt/trn_rl_repo",):
    if _p not in sys.path:
        sys.path.insert(0, _p)

import ml_dtypes  # noqa: E402

B, T, D, H, C = 256, 1024, 64, 128, 10
NCORES, BL = 8, 32
# gate-block order [i, f, o, g]; reference split order is (i, f, g, o)
PERM = [0, 1, 3, 2]
LAGS = 8  # steps that layer 1 trails layer 0

_cache = {}


def _build_nc(t_steps):
    from contextlib import ExitStack

    import concourse.bass as bass
    import concourse.mybir as mybir
    from concourse import bacc
    from concourse.tile import TileContext

    dt = mybir.dt
    AF = mybir.ActivationFunctionType
    MS = bass.MemorySpace

    nc = bacc.Bacc(None, target_bir_lowering=False, debug=False)
    NP = t_steps + LAGS  # pairs
    NBANK = NP // 2

    xta_d = nc.dram_tensor("xta", [D + 1, t_steps * BL], dt.bfloat16, kind="ExternalInput")
    w0aug_d = nc.dram_tensor("w0aug", [D + 1, 512], dt.bfloat16, kind="ExternalInput")
    whh0_d = nc.dram_tensor("whh0t", [H, 512], dt.bfloat16, kind="ExternalInput")
    w1_d = nc.dram_tensor("w1t", [H, 512], dt.bfloat16, kind="ExternalInput")
    whh1_d = nc.dram_tensor("whh1t", [H, 512], dt.bfloat16, kind="ExternalInput")
    b1_d = nc.dram_tensor("b1row", [4, H], dt.bfloat16, kind="ExternalInput")
    ind_d = nc.dram_tensor("ind", [4, 512], dt.bfloat16, kind="ExternalInput")
    whead_d = nc.dram_tensor("wheadt", [H, 16], dt.bfloat16, kind="ExternalInput")
    bhead_d = nc.dram_tensor("bhead", [16, 1], dt.float32, kind="ExternalInput")
    out_d = nc.dram_tensor("out", [16, BL], dt.float32, kind="ExternalOutput")

    with TileContext(nc) as tc, ExitStack() as ctx:
        consts = ctx.enter_context(tc.tile_pool(name="consts", bufs=1))
        xta = consts.tile([D + 1, t_steps * BL], dt.bfloat16, tag="xta")
        w0aug = consts.tile([D + 1, 512], dt.bfloat16, tag="w0aug")
        whh0 = consts.tile([H, 512], dt.bfloat16, tag="whh0")
        w1 = consts.tile([H, 512], dt.bfloat16, tag="w1")
        whh1 = consts.tile([H, 512], dt.bfloat16, tag="whh1")
        b1row = consts.tile([4, H], dt.bfloat16, tag="b1row")
        ind = consts.tile([4, 512], dt.bfloat16, tag="ind")
        wheadt = consts.tile([H, 16], dt.bfloat16, tag="wheadt")
        bhead = consts.tile([16, 1], dt.float32, tag="bhead")
        h1T = consts.tile([H, t_steps, BL], dt.bfloat16, tag="h1T")
        # htmp[:, 0:32] = layer0 h (current), [:, 32:64] = layer1 h
        htmp = consts.tile([H, 2 * BL], dt.bfloat16, tag="htmp")
        cc = consts.tile([H, 2 * BL], dt.float32, tag="cc")

        # input DMAs (xta split so the first GEMMs can start early)
        nxc = 8
        csz = (t_steps * BL) // nxc
        for i in range(nxc):
            nc.sync.dma_start(xta[:, i * csz:(i + 1) * csz], xta_d[:, i * csz:(i + 1) * csz])
        for tl, dr in ((w0aug, w0aug_d), (whh0, whh0_d), (w1, w1_d), (whh1, whh1_d),
                       (b1row, b1_d), (ind, ind_d), (wheadt, whead_d), (bhead, bhead_d)):
            nc.sync.dma_start(tl[:], dr[:])
        nc.vector.memset(htmp[:], 0.0)
        nc.vector.memset(cc[:], 0.0)

        psum = ctx.enter_context(tc.tile_pool(name="psum", bufs=3, space=MS.PSUM))
        psumh = ctx.enter_context(tc.tile_pool(name="psumh", bufs=1, space=MS.PSUM))
        sp = ctx.enter_context(tc.tile_pool(name="sp", bufs=4))
        tp = ctx.enter_context(tc.tile_pool(name="tp", bufs=4))

        bank_of = {}

        # bank layout: col = (pair%2)*256 + gate*64 + layer*32 + b
        def gemm(k):
            # bank k serves pairs (2k, 2k+1); L0 steps (2k, 2k+1),
            # L1 steps (2k-LAGS, 2k+1-LAGS)
            has_a = 2 * k < t_steps
            has_b = 2 * k >= LAGS
            bank = psum.tile([H, 512], dt.float32, tag="bank")
            bank_of[k] = bank
            if k - 2 in bank_of:
                del bank_of[k - 2]
            bv = bank[:].rearrange("p (t j w x) -> p t j w x", t=2, j=4, w=2)
            if has_b:
                # bias everywhere on L1 cols; zeros elsewhere; clears bank
                nc.tensor.matmul(bank[:], b1row[:], ind[:], start=True, stop=False)
            if has_a:
                rhs = xta[:, 2 * k * BL:(2 * k + 2) * BL]
                for j in range(4):
                    nc.tensor.matmul(bv[:, :, j, 0, :], w0aug[:, j * H:(j + 1) * H],
                                     rhs, start=(not has_b and j == 0), stop=False)
            if has_b:
                tb = 2 * k - LAGS
                rhs = h1T[:, tb:tb + 2, :]
                for j in range(4):
                    nc.tensor.matmul(bv[:, :, j, 1, :], w1[:, j * H:(j + 1) * H],
                                     rhs, start=False, stop=False)

        # sig layout: [sig_i(0:64) | sig_f(64:128) | sig_o(128:192) | tanh_g(192:256)]
        # each 64-block = [layerA 32 | layerB 32]
        def pair_full(m):
            bank = bank_of[m // 2]
            base = (m % 2) * 256
            for j in range(4):
                nc.tensor.matmul(bank[:, base + j * 64:base + j * 64 + 32],
                                 whh0[:, j * H:(j + 1) * H], htmp[:, 0:BL],
                                 start=False, stop=True)
                nc.tensor.matmul(bank[:, base + j * 64 + 32:base + (j + 1) * 64],
                                 whh1[:, j * H:(j + 1) * H], htmp[:, BL:2 * BL],
                                 start=False, stop=True)
            sig = sp.tile([H, 256], dt.float32, tag="sig")
            nc.scalar.activation(sig[:, 0:128], bank[:, base:base + 128], AF.Sigmoid)
            nc.scalar.activation(sig[:, 192:256], bank[:, base + 192:base + 256], AF.Tanh)
            nc.scalar.activation(sig[:, 128:192], bank[:, base + 128:base + 192], AF.Sigmoid)
            tmp = tp.tile([H, 2 * BL], dt.float32, tag="tmp")
            nc.vector.tensor_mul(cc[:], sig[:, 64:128], cc[:])
            nc.vector.tensor_mul(tmp[:], sig[:, 0:64], sig[:, 192:256])
            nc.vector.tensor_add(cc[:], cc[:], tmp[:])
            th = tp.tile([H, 2 * BL], dt.float32, tag="th")
            nc.scalar.activation(th[:], cc[:], AF.Tanh)
            nc.vector.tensor_mul(htmp[:], sig[:, 128:192], th[:])
            nc.vector.tensor_copy(h1T[:, m, :], htmp[:, 0:BL])

        def pair_half(m, w):
            # w = 0: layer0 only (m < LAGS); w = 1: layer1 only (m >= t_steps)
            bank = bank_of[m // 2]
            region = (m % 2) * 256
            base = region + w * 32
            csl = slice(w * BL, (w + 1) * BL)
            whh = whh0 if w == 0 else whh1
            for j in range(4):
                nc.tensor.matmul(bank[:, base + j * 64:base + j * 64 + 32],
                                 whh[:, j * H:(j + 1) * H], htmp[:, csl],
                                 start=False, stop=True)
            sig = sp.tile([H, 256], dt.float32, tag="sig")
            bv = bank[:, region:region + 256].rearrange("p (j x) -> p j x", j=4)
            sv = sig[:].rearrange("p (j x) -> p j x", j=4)
            ws = slice(w * 32, (w + 1) * 32)
            nc.scalar.activation(sv[:, 0:2, ws], bv[:, 0:2, ws], AF.Sigmoid)
            nc.scalar.activation(sv[:, 3, ws], bv[:, 3, ws], AF.Tanh)
            nc.scalar.activation(sv[:, 2, ws], bv[:, 2, ws], AF.Sigmoid)
            tmp = tp.tile([H, 2 * BL], dt.float32, tag="tmp")
            nc.vector.tensor_mul(cc[:, csl], sig[:, 64 + w * 32:64 + (w + 1) * 32], cc[:, csl])
            nc.vector.tensor_mul(tmp[:, csl], sig[:, w * 32:(w + 1) * 32],
                                 sig[:, 192 + w * 32:192 + (w + 1) * 32])
            nc.vector.tensor_add(cc[:, csl], cc[:, csl], tmp[:, csl])
            th = tp.tile([H, 2 * BL], dt.float32, tag="th")
            nc.scalar.activation(th[:, csl], cc[:, csl], AF.Tanh)
            nc.vector.tensor_mul(htmp[:, csl], sig[:, 128 + w * 32:128 + (w + 1) * 32], th[:, csl])
            if w == 0:
                nc.vector.tensor_copy(h1T[:, m, :], htmp[:, 0:BL])

        gemm(0)
        for m in range(NP):
            if m < LAGS:
                pair_half(m, 0)
            elif m >= t_steps:
                pair_half(m, 1)
            else:
                pair_full(m)
            # prefetch the next bank's xg GEMM one full pair ahead
            if m % 2 == 0 and m // 2 + 1 < NBANK:
                gemm(m // 2 + 1)

        hp = psumh.tile([16, BL], dt.float32, tag="head")
        nc.tensor.matmul(hp[:], wheadt[:], htmp[:, BL:2 * BL], start=True, stop=True)
        outs = consts.tile([16, BL], dt.float32, tag="outs")
        nc.scalar.activation(outs[:], hp[:], AF.Identity, bias=bhead[:, 0:1])
        nc.sync.dma_start(out_d[:], outs[:])

    nc.compile()
    return nc


def _pack_shared(W_ih0, W_hh0, b_ih0, b_hh0, W_ih1, W_hh1, b_ih1, b_hh1, W_head, b_head):
    bf16 = ml_dtypes.bfloat16
    b0 = (b_ih0 + b_hh0).astype(np.float32)
    b1 = (b_ih1 + b_hh1).astype(np.float32)

    w0aug = np.zeros((D + 1, 512), np.float32)
    whh0t = np.zeros((H, 512), np.float32)
    w1t = np.zeros((H, 512), np.float32)
    whh1t = np.zeros((H, 512), np.float32)
    b1row = np.zeros((4, H), np.float32)
    for j, g in enumerate(PERM):
        sl = slice(g * H, (g + 1) * H)
        w0aug[:D, j * H:(j + 1) * H] = W_ih0[sl].T
        w0aug[D, j * H:(j + 1) * H] = b0[sl]
        whh0t[:, j * H:(j + 1) * H] = W_hh0[sl].T
        w1t[:, j * H:(j + 1) * H] = W_ih1[sl].T
        whh1t[:, j * H:(j + 1) * H] = W_hh1[sl].T
        b1row[j] = b1[sl]

    # bias indicator: 1 on layer-1 columns of gate-block j
    # col = (pair%2)*256 + j*64 + layer*32 + b
    ind = np.zeros((4, 512), np.float32)
    cols = np.arange(512)
    for r in range(4):
        ind[r] = (((cols % 256) // 64 == r) & ((cols % 64) // 32 == 1)).astype(np.float32)

    wheadt = np.zeros((H, 16), np.float32)
    wheadt[:, :C] = W_head.T
    bhead = np.zeros((16, 1), np.float32)
    bhead[:C, 0] = b_head

    return {
        "w0aug": w0aug.astype(bf16), "whh0t": whh0t.astype(bf16),
        "w1t": w1t.astype(bf16), "whh1t": whh1t.astype(bf16),
        "b1row": b1row.astype(bf16), "ind": ind.astype(bf16),
        "wheadt": wheadt.astype(bf16), "bhead": bhead.astype(np.float32),
    }


def _make_xta(x_core, t_steps):
    # x_core [BL, T, D] -> [D+1, T*BL] with ones row (bias lane)
    bf16 = ml_dtypes.bfloat16
    xt = x_core[:, :t_steps, :].transpose(2, 1, 0).reshape(D, t_steps * BL)
    out = np.ones((D + 1, t_steps * BL), np.float32)
    out[:D] = xt
    return out.astype(bf16)


def run_cores(x, weights, t_steps=T, trace=False):
    from concourse.bass_utils import run_bass_kernel_spmd

    key = t_steps
    if key not in _cache:
        _cache[key] = _build_nc(t_steps)
    nc = _cache[key]

    shared = _pack_shared(**weights)
    in_maps = []
    for i in range(NCORES):
        m = dict(shared)
        m["xta"] = _make_xta(x[i * BL:(i + 1) * BL], t_steps)
        in_maps.append(m)
    res = run_bass_kernel_spmd(nc, in_maps, list(range(NCORES)), trace=trace)
    out = np.zeros((B, C), np.float32)
    for i in range(NCORES):
        out[i * BL:(i + 1) * BL] = res.results[i]["out"][:C, :].T
    return out, res


def kernel(x, W_ih0, W_hh0, b_ih0, b_hh0, W_ih1, W_hh1, b_ih1, b_hh1, W_head, b_head):
    weights = dict(W_ih0=W_ih0, W_hh0=W_hh0, b_ih0=b_ih0, b_hh0=b_hh0,
                   W_ih1=W_ih1, W_hh1=W_hh1, b_ih1=b_ih1, b_hh1=b_hh1,
                   W_head=W_head, b_head=b_head)
    weights = {k: np.asarray(v, np.float32) for k, v in weights.items()}
    out, _ = run_cores(np.asarray(x, np.float32), weights)
    return out


# revision 14
# speedup vs baseline: 1.1539x; 1.1539x over previous
"""Trainium2 Bass kernel for nn_LSTMClassifier (B=256,T=1024,D=64,H=128,C=10).

Strategy: data-parallel over batch across 8 cores (32 seqs/core).
Per-core layout is gate-major: partitions = hidden units, batch on the
free dim, so h^T [128,32] is born in the right layout to be the moving
operand of the next step's gate matmuls (no per-step transposes).

The two LSTM layers are MERGED into one instruction stream: at "pair"
m, layer 0 runs step t=m while layer 1 runs step t=m-LAGS, and every
activation / vector op covers both layers in a single instruction
(per-instruction fixed cost on ACT ~180-370ns and DVE ~155-225ns dwarfs
the per-element cost).

Gates are SPLIT ACROSS TWO PSUM BANKS so the critical sigmoid(i,f)
only waits on its own bank's accumulation group (4 matmuls), not all
8: bank P holds gates (i,f), bank Q holds (o,g), each covering FOUR
pairs:  col = (m%4)*128 + gate_local*64 + layer*32 + b.

Per layer the input transform xg = W_ih @ x + b is computed by chunked
GEMMs directly INTO the PSUM banks that the per-step recurrence matmuls
then accumulate onto (start=False). L0's bias rides a K=65 augmented
stationary (ones row in x^T); L1's bias is one indicator matmul per
bank (doubling as the bank's start=True clear).

sigma(o) and tanh(c) are written bf16 so the final h multiply runs in
the DVE 2x 16-bit mode (~136ns vs ~226ns).
"""

import os
import sys

import numpy as np

for _p in ("/opt/trn_rl_repo",):
    if _p not in sys.path:
        sys.path.insert(0, _p)

import ml_dtypes  # noqa: E402

B, T, D, H, C = 256, 1024, 64, 128, 10
NCORES, BL = 8, 32
# gate-block order [i, f, o, g]; reference split order is (i, f, g, o)
PERM = [0, 1, 3, 2]
LAGS = 8  # steps that layer 1 trails layer 0 (must be multiple of 4)

_cache = {}


def _build_nc(t_steps):
    from contextlib import ExitStack

    import concourse.bass as bass
    import concourse.mybir as mybir
    from concourse import bacc
    from concourse.tile import TileContext

    dt = mybir.dt
    AF = mybir.ActivationFunctionType
    MS = bass.MemorySpace

    nc = bacc.Bacc(None, target_bir_lowering=False, debug=False)
    NP = t_steps + LAGS  # pairs
    NQUAD = NP // 4

    xta_d = nc.dram_tensor("xta", [D + 1, t_steps * BL], dt.bfloat16, kind="ExternalInput")
    w0aug_d = nc.dram_tensor("w0aug", [D + 1, 512], dt.bfloat16, kind="ExternalInput")
    whh0_d = nc.dram_tensor("whh0t", [H, 512], dt.bfloat16, kind="ExternalInput")
    w1_d = nc.dram_tensor("w1t", [H, 512], dt.bfloat16, kind="ExternalInput")
    whh1_d = nc.dram_tensor("whh1t", [H, 512], dt.bfloat16, kind="ExternalInput")
    b1_d = nc.dram_tensor("b1row", [4, H], dt.bfloat16, kind="ExternalInput")
    indp_d = nc.dram_tensor("indp", [4, 512], dt.bfloat16, kind="ExternalInput")
    indq_d = nc.dram_tensor("indq", [4, 512], dt.bfloat16, kind="ExternalInput")
    whead_d = nc.dram_tensor("wheadt", [H, 16], dt.bfloat16, kind="ExternalInput")
    bhead_d = nc.dram_tensor("bhead", [16, 1], dt.float32, kind="ExternalInput")
    out_d = nc.dram_tensor("out", [16, BL], dt.float32, kind="ExternalOutput")

    with TileContext(nc) as tc, ExitStack() as ctx:
        consts = ctx.enter_context(tc.tile_pool(name="consts", bufs=1))
        xta = consts.tile([D + 1, t_steps * BL], dt.bfloat16, tag="xta")
        w0aug = consts.tile([D + 1, 512], dt.bfloat16, tag="w0aug")
        whh0 = consts.tile([H, 512], dt.bfloat16, tag="whh0")
        w1 = consts.tile([H, 512], dt.bfloat16, tag="w1")
        whh1 = consts.tile([H, 512], dt.bfloat16, tag="whh1")
        b1row = consts.tile([4, H], dt.bfloat16, tag="b1row")
        indp = consts.tile([4, 512], dt.bfloat16, tag="indp")
        indq = consts.tile([4, 512], dt.bfloat16, tag="indq")
        wheadt = consts.tile([H, 16], dt.bfloat16, tag="wheadt")
        bhead = consts.tile([16, 1], dt.float32, tag="bhead")
        h1T = consts.tile([H, t_steps, BL], dt.bfloat16, tag="h1T")
        # htmp[:, 0:32] = layer0 h (current), [:, 32:64] = layer1 h
        htmp = consts.tile([H, 2 * BL], dt.bfloat16, tag="htmp")
        cc = consts.tile([H, 2 * BL], dt.float32, tag="cc")

        # input DMAs (xta split so the first GEMMs can start early)
        nxc = 8
        csz = (t_steps * BL) // nxc
        for i in range(nxc):
            nc.sync.dma_start(xta[:, i * csz:(i + 1) * csz], xta_d[:, i * csz:(i + 1) * csz])
        for tl, dr in ((w0aug, w0aug_d), (whh0, whh0_d), (w1, w1_d), (whh1, whh1_d),
                       (b1row, b1_d), (indp, indp_d), (indq, indq_d),
                       (wheadt, whead_d), (bhead, bhead_d)):
            nc.sync.dma_start(tl[:], dr[:])
        nc.vector.memset(htmp[:], 0.0)
        nc.vector.memset(cc[:], 0.0)

        psump = ctx.enter_context(tc.tile_pool(name="psump", bufs=3, space=MS.PSUM))
        psumq = ctx.enter_context(tc.tile_pool(name="psumq", bufs=3, space=MS.PSUM))
        psumh = ctx.enter_context(tc.tile_pool(name="psumh", bufs=1, space=MS.PSUM))
        sp = ctx.enter_context(tc.tile_pool(name="sp", bufs=4))
        tp = ctx.enter_context(tc.tile_pool(name="tp", bufs=4))

        bank_of = {}

        # P bank: gates (i,f); Q bank: gates (o,g); each serves 4 pairs.
        # col = (pair%4)*128 + gate_local*64 + layer*32 + b
        def gemm(k):
            has_a = 4 * k < t_steps
            has_b = 4 * k >= LAGS
            bp = psump.tile([H, 512], dt.float32, tag="bankp")
            bq = psumq.tile([H, 512], dt.float32, tag="bankq")
            bank_of[k] = (bp, bq)
            bank_of.pop(k - 2, None)
            rhs_a = xta[:, 4 * k * BL:(4 * k + 4) * BL] if has_a else None
            tb = 4 * k - LAGS
            rhs_b = h1T[:, tb:tb + 4, :] if has_b else None
            for bank, ind, jg in ((bp, indp, (0, 1)), (bq, indq, (2, 3))):
                bv = bank[:].rearrange("p (t j w x) -> p t j w x", t=4, j=2, w=2)
                if has_b:
                    nc.tensor.matmul(bank[:], b1row[:], ind[:], start=True, stop=False)
                if has_a:
                    for jl, j in enumerate(jg):
                        nc.tensor.matmul(bv[:, :, jl, 0, :], w0aug[:, j * H:(j + 1) * H],
                                         rhs_a, start=(not has_b and jl == 0), stop=False)
                if has_b:
                    for jl, j in enumerate(jg):
                        nc.tensor.matmul(bv[:, :, jl, 1, :], w1[:, j * H:(j + 1) * H],
                                         rhs_b, start=False, stop=False)

        # sig layout: [sig_i(0:64) | sig_f(64:128) | tanh_g(128:192)], each
        # 64-block = [layerA 32 | layerB 32]; sig_o and tanh_c go to bf16
        # tiles so the h multiply runs in DVE 2x mode.
        def mm8(m, w_only=None):
            bp, bq = bank_of[m // 4]
            base = (m % 4) * 128
            ws = (0, 1) if w_only is None else (w_only,)
            for bank, j0 in ((bp, 0), (bq, 2)):
                for jl in range(2):
                    j = j0 + jl  # global gate block: 0=i 1=f 2=o 3=g
                    for w in ws:
                        whh = whh0 if w == 0 else whh1
                        nc.tensor.matmul(
                            bank[:, base + jl * 64 + w * 32:base + jl * 64 + w * 32 + 32],
                            whh[:, j * H:(j + 1) * H],
                            htmp[:, w * BL:(w + 1) * BL], start=False, stop=True)

        def pair_full(m):
            bp, bq = bank_of[m // 4]
            base = (m % 4) * 128
            mm8(m)
            sig = sp.tile([H, 192], dt.float32, tag="sig")
            sob = tp.tile([H, 2 * BL], dt.bfloat16, tag="sob")
            nc.scalar.activation(sig[:, 0:128], bp[:, base:base + 128], AF.Sigmoid)
            nc.scalar.activation(sig[:, 128:192], bq[:, base + 64:base + 128], AF.Tanh)
            nc.scalar.activation(sob[:], bq[:, base:base + 64], AF.Sigmoid)
            tmp = tp.tile([H, 2 * BL], dt.float32, tag="tmp")
            nc.vector.tensor_mul(cc[:], sig[:, 64:128], cc[:])
            nc.vector.tensor_mul(tmp[:], sig[:, 0:64], sig[:, 128:192])
            nc.vector.tensor_add(cc[:], cc[:], tmp[:])
            th = tp.tile([H, 2 * BL], dt.bfloat16, tag="th")
            nc.scalar.activation(th[:], cc[:], AF.Tanh)
            nc.vector.tensor_mul(htmp[:], sob[:], th[:])
            nc.vector.tensor_copy(h1T[:, m, :], htmp[:, 0:BL])

        def pair_half(m, w):
            # w = 0: layer0 only (m < LAGS); w = 1: layer1 only (m >= t_steps)
            bp, bq = bank_of[m // 4]
            base = (m % 4) * 128
            csl = slice(w * BL, (w + 1) * BL)
            mm8(m, w_only=w)
            sig = sp.tile([H, 192], dt.float32, tag="sig")
            sob = tp.tile([H, 2 * BL], dt.bfloat16, tag="sob")
            pv = bp[:, base:base + 128].rearrange("p (j x) -> p j x", j=2)
            sv = sig[:, 0:128].rearrange("p (j x) -> p j x", j=2)
            wsl = slice(w * 32, (w + 1) * 32)
            nc.scalar.activation(sv[:, :, wsl], pv[:, :, wsl], AF.Sigmoid)
            nc.scalar.activation(sig[:, 128 + w * 32:128 + (w + 1) * 32],
                                 bq[:, base + 64 + w * 32:base + 64 + (w + 1) * 32], AF.Tanh)
            nc.scalar.activation(sob[:, csl], bq[:, base + w * 32:base + (w + 1) * 32],
                                 AF.Sigmoid)
            tmp = tp.tile([H, 2 * BL], dt.float32, tag="tmp")
            nc.vector.tensor_mul(cc[:, csl], sig[:, 64 + w * 32:64 + (w + 1) * 32], cc[:, csl])
            nc.vector.tensor_mul(tmp[:, csl], sig[:, wsl],
                                 sig[:, 128 + w * 32:128 + (w + 1) * 32])
            nc.vector.tensor_add(cc[:, csl], cc[:, csl], tmp[:, csl])
            th = tp.tile([H, 2 * BL], dt.bfloat16, tag="th")
            nc.scalar.activation(th[:, csl], cc[:, csl], AF.Tanh)
            nc.vector.tensor_mul(htmp[:, csl], sob[:, csl], th[:, csl])
            if w == 0:
                nc.vector.tensor_copy(h1T[:, m, :], htmp[:, 0:BL])

        gemm(0)
        for m in range(NP):
            if m < LAGS:
                pair_half(m, 0)
            elif m >= t_steps:
                pair_half(m, 1)
            else:
                pair_full(m)
            # prefetch the next quad's xg GEMM with ~3 pairs of lead
            if m % 4 == 1 and m // 4 + 1 < NQUAD:
                gemm(m // 4 + 1)

        hp = psumh.tile([16, BL], dt.float32, tag="head")
        nc.tensor.matmul(hp[:], wheadt[:], htmp[:, BL:2 * BL], start=True, stop=True)
        outs = consts.tile([16, BL], dt.float32, tag="outs")
        nc.scalar.activation(outs[:], hp[:], AF.Identity, bias=bhead[:, 0:1])
        nc.sync.dma_start(out_d[:], outs[:])

    nc.compile()
    return nc


def _pack_shared(W_ih0, W_hh0, b_ih0, b_hh0, W_ih1, W_hh1, b_ih1, b_hh1, W_head, b_head):
    bf16 = ml_dtypes.bfloat16
    b0 = (b_ih0 + b_hh0).astype(np.float32)
    b1 = (b_ih1 + b_hh1).astype(np.float32)

    w0aug = np.zeros((D + 1, 512), np.float32)
    whh0t = np.zeros((H, 512), np.float32)
    w1t = np.zeros((H, 512), np.float32)
    whh1t = np.zeros((H, 512), np.float32)
    b1row = np.zeros((4, H), np.float32)
    for j, g in enumerate(PERM):
        sl = slice(g * H, (g + 1) * H)
        w0aug[:D, j * H:(j + 1) * H] = W_ih0[sl].T
        w0aug[D, j * H:(j + 1) * H] = b0[sl]
        whh0t[:, j * H:(j + 1) * H] = W_hh0[sl].T
        w1t[:, j * H:(j + 1) * H] = W_ih1[sl].T
        whh1t[:, j * H:(j + 1) * H] = W_hh1[sl].T
        b1row[j] = b1[sl]

    # bias indicators: 1 on layer-1 columns of each bank's gate blocks.
    # P holds gate blocks (0,1)=(i,f); Q holds (2,3)=(o,g).
    # col = (pair%4)*128 + gate_local*64 + layer*32 + b
    cols = np.arange(512)
    gl = (cols % 128) // 64
    lay = (cols % 64) // 32
    indp = np.zeros((4, 512), np.float32)
    indq = np.zeros((4, 512), np.float32)
    for r in range(2):
        indp[r] = ((gl == r) & (lay == 1)).astype(np.float32)
        indq[2 + r] = ((gl == r) & (lay == 1)).astype(np.float32)

    wheadt = np.zeros((H, 16), np.float32)
    wheadt[:, :C] = W_head.T
    bhead = np.zeros((16, 1), np.float32)
    bhead[:C, 0] = b_head

    return {
        "w0aug": w0aug.astype(bf16), "whh0t": whh0t.astype(bf16),
        "w1t": w1t.astype(bf16), "whh1t": whh1t.astype(bf16),
        "b1row": b1row.astype(bf16), "indp": indp.astype(bf16),
        "indq": indq.astype(bf16),
        "wheadt": wheadt.astype(bf16), "bhead": bhead.astype(np.float32),
    }


def _make_xta(x_core, t_steps):
    # x_core [BL, T, D] -> [D+1, T*BL] with ones row (bias lane)
    bf16 = ml_dtypes.bfloat16
    xt = x_core[:, :t_steps, :].transpose(2, 1, 0).reshape(D, t_steps * BL)
    out = np.ones((D + 1, t_steps * BL), np.float32)
    out[:D] = xt
    return out.astype(bf16)


def run_cores(x, weights, t_steps=T, trace=False):
    from concourse.bass_utils import run_bass_kernel_spmd

    key = t_steps
    if key not in _cache:
        _cache[key] = _build_nc(t_steps)
    nc = _cache[key]

    shared = _pack_shared(**weights)
    in_maps = []
    for i in range(NCORES):
        m = dict(shared)
        m["xta"] = _make_xta(x[i * BL:(i + 1) * BL], t_steps)
        in_maps.append(m)
    res = run_bass_kernel_spmd(nc, in_maps, list(range(NCORES)), trace=trace)
    out = np.zeros((B, C), np.float32)
    for i in range(NCORES):
        out[i * BL:(i + 1) * BL] = res.results[i]["out"][:C, :].T
    return out, res


def kernel(x, W_ih0, W_hh0, b_ih0, b_hh0, W_ih1, W_hh1, b_ih1, b_hh1, W_head, b_head):
    weights = dict(W_ih0=W_ih0, W_hh0=W_hh0, b_ih0=b_ih0, b_hh0=b_hh0,
                   W_ih1=W_ih1, W_hh1=W_hh1, b_ih1=b_ih1, b_hh1=b_hh1,
                   W_head=W_head, b_head=b_head)
    weights = {k: np.asarray(v, np.float32) for k, v in weights.items()}
    out, _ = run_cores(np.asarray(x, np.float32), weights)
    return out
